# revision 26
# baseline (speedup 1.0000x reference)
"""Trainium2 Bass kernel for EquiLocalPatOrientConvolution.

Reference computes: filt[o,i,·] = Y0 * sum_p W[o,i,p] * radial_mask_p (5x5x5),
then conv3d(x, filt, pad=2) + bias.

Factorized device algorithm (11x fewer MACs than direct conv):
  stage 1 (DVE, fp16): per input channel, radial-shell sums of x via a
    separable symmetric-shift hierarchy (w-axis sums -> 2D shells over (h,w)
    -> 3D shells), ~31 ops/point instead of 125 taps.
  stage 2 (PE): out[o,n] = sum_{i,p} (Y0*W[o,i,p]) * y[i,p,n] as K=320
    contraction (3 PSUM-accumulated matmuls), bias fused in the PSUM->SBUF
    activation copy.

Sharding: 8 cores = (batch 2) x (depth quarters 4); each core gets a
zero-padded x slab [32, 16, 52, 52] (12 depth + 2+2 halo, h/w padded).
"""

import numpy as np

import concourse.bacc as bacc
import concourse.mybir as mybir
from concourse.tile import TileContext
from concourse.bass_utils import run_bass_kernel_spmd

Y0 = 0.28209479177387814
F16 = mybir.dt.float16
F32 = mybir.dt.float32

_PROG_CACHE = {}


def _build_program():
    nc = bacc.Bacc(None)
    xs = nc.declare_dram_parameter("xs", [32, 16, 52, 52], F16, isOutput=False)
    w0 = nc.declare_dram_parameter("w0", [128, 64], F16, isOutput=False)
    w1 = nc.declare_dram_parameter("w1", [128, 64], F16, isOutput=False)
    w2 = nc.declare_dram_parameter("w2", [64, 64], F16, isOutput=False)
    bias = nc.declare_dram_parameter("bias", [64, 1], F32, isOutput=False)
    out_d = nc.declare_dram_parameter("out", [64, 12, 48, 48], F32, isOutput=True)

    V = nc.vector
    AF = mybir.ActivationFunctionType

    with TileContext(nc) as tc:
        with (
            tc.tile_pool(name="persist", bufs=1) as pp,
            tc.tile_pool(name="pspool", bufs=4, space="PSUM") as psp,
        ):
            # ---- load x slab: partitions = (hc, i) = hc*32 + i ----
            x_sb = pp.tile([128, 16, 16, 52], F16)
            for hc in range(4):
                eng = nc.sync if hc % 2 == 0 else nc.scalar
                eng.dma_start(
                    out=x_sb[hc * 32 : (hc + 1) * 32],
                    in_=xs[:, :, hc * 12 : hc * 12 + 16, :],
                )
            w_sb0 = pp.tile([128, 64], F16)
            w_sb1 = pp.tile([128, 64], F16)
            w_sb2 = pp.tile([64, 64], F16)
            nc.sync.dma_start(out=w_sb0[:], in_=w0[:])
            nc.sync.dma_start(out=w_sb1[:], in_=w1[:])
            nc.sync.dma_start(out=w_sb2[:], in_=w2[:])
            w_chunks = [w_sb0, w_sb1, w_sb2]
            bias_sb = pp.tile([64, 1], F32)
            nc.sync.dma_start(out=bias_sb[:], in_=bias[:])

            # z tiles outlive the u scope below
            z1 = pp.tile([128, 16, 12, 48], F16)
            z2 = pp.tile([128, 16, 12, 48], F16)
            z4 = pp.tile([128, 16, 12, 48], F16)
            z5 = pp.tile([128, 16, 12, 48], F16)
            z8 = pp.tile([128, 16, 12, 48], F16)

            # ---- stage 1a/1b in a transient pool scope (u freed after) ----
            with tc.tile_pool(name="upool", bufs=1) as up:
                # w-axis symmetric sums u_s = x(w-s) + x(w+s)
                u1 = up.tile([128, 16, 16, 48], F16)
                u2 = up.tile([128, 16, 16, 48], F16)
                V.tensor_add(out=u1[:], in0=x_sb[:, :, :, 1:49], in1=x_sb[:, :, :, 3:51])
                V.tensor_add(out=u2[:], in0=x_sb[:, :, :, 0:48], in1=x_sb[:, :, :, 4:52])

                # 2D shells over (h,w): z_A, A in {0,1,2,4,5,8}
                # z0 = x (view); others at h-interior [2,14), w-interior
                V.tensor_add(out=z1[:], in0=x_sb[:, :, 1:13, 2:50], in1=x_sb[:, :, 3:15, 2:50])
                V.tensor_add(out=z1[:], in0=z1[:], in1=u1[:, :, 2:14, :])
                nc.gpsimd.tensor_add(out=z2[:], in0=u1[:, :, 1:13, :], in1=u1[:, :, 3:15, :])
                V.tensor_add(out=z4[:], in0=x_sb[:, :, 0:12, 2:50], in1=x_sb[:, :, 4:16, 2:50])
                V.tensor_add(out=z4[:], in0=z4[:], in1=u2[:, :, 2:14, :])
                V.tensor_add(out=z5[:], in0=u1[:, :, 0:12, :], in1=u1[:, :, 4:16, :])
                V.tensor_add(out=z5[:], in0=z5[:], in1=u2[:, :, 1:13, :])
                V.tensor_add(out=z5[:], in0=z5[:], in1=u2[:, :, 3:15, :])
                nc.gpsimd.tensor_add(out=z8[:], in0=u2[:, :, 0:12, :], in1=u2[:, :, 4:16, :])

            def x0(d):
                return x_sb[:, d, 2:14, 2:50]

            nsl = [(0, 512), (512, 512), (1024, 512), (1536, 512), (2048, 256)]

            # ---- per output-depth plane: 3D shells -> regroup -> matmul ----
            with (
                tc.tile_pool(name="ypool", bufs=2) as yp,
                tc.tile_pool(name="y2pool", bufs=2) as y2p,
                tc.tile_pool(name="outpool", bufs=2) as outp,
            ):
                _run_planes(nc, V, x0, x_sb, z1, z2, z4, z5, z8, yp, y2p, psp, outp,
                            w_chunks, bias_sb, out_d, nsl)
    # Bacc finalize runs generate_event_semaphores: splits embedded waits
    # into standalone EventSemaphore insts (walrus rejects >1 wait per inst)
    nc.finalize()
    return nc


def _run_planes(nc, V, x0, x_sb, z1, z2, z4, z5, z8, yp, y2p, psp, outp,
                w_chunks, bias_sb, out_d, nsl):
    AF = mybir.ActivationFunctionType
    if True:
        if True:
            for dp in range(12):
                d = dp + 2
                y = yp.tile([128, 10, 12, 48], F16, tag="y")
                y2 = [
                    y2p.tile([128, 2304], F16, tag="y2_0", name="y2_0"),
                    y2p.tile([128, 2304], F16, tag="y2_1", name="y2_1"),
                    y2p.tile([64, 2304], F16, tag="y2_2", name="y2_2"),
                ]

                def regroup(c, plo, phi):
                    # [(hc,i), (p,h,w)] -> K-chunk layout [(i,p), n];
                    # chunk rows = i*4 + p_local (c<2), i*2 + p_local (c=2);
                    # free n = (hc, h, w) flattened to 2304. Issued right
                    # after the producing ops so chunk-c matmuls overlap
                    # the DVE work of later shells.
                    for hc in range(4):
                        nc.sync.dma_start(
                            out=y2[c][:, hc * 576 : (hc + 1) * 576],
                            in_=y[hc * 32 : (hc + 1) * 32, plo:phi],
                        )

                # p0 (r2=0) on ScalarE (idle) to keep DVE for the adds
                nc.scalar.activation(y[:, 0], x0(d), AF.Copy)
                # p1 (r2=1) = D1 z0 + z1
                V.tensor_add(out=y[:, 1], in0=x0(d - 1), in1=x0(d + 1))
                V.tensor_add(out=y[:, 1], in0=y[:, 1], in1=z1[:, d])
                # p2 (r2=2) = D1 z1 + z2
                V.tensor_add(out=y[:, 2], in0=z1[:, d - 1], in1=z1[:, d + 1])
                V.tensor_add(out=y[:, 2], in0=y[:, 2], in1=z2[:, d])
                # p3 (r2=3) = D1 z2
                V.tensor_add(out=y[:, 3], in0=z2[:, d - 1], in1=z2[:, d + 1])
                regroup(0, 0, 4)
                # p4 (r2=4) = D2 z0 + z4
                V.tensor_add(out=y[:, 4], in0=x0(d - 2), in1=x0(d + 2))
                V.tensor_add(out=y[:, 4], in0=y[:, 4], in1=z4[:, d])
                # p5 (r2=5) = D2 z1 + D1 z4 + z5
                V.tensor_add(out=y[:, 5], in0=z1[:, d - 2], in1=z1[:, d + 2])
                V.tensor_add(out=y[:, 5], in0=y[:, 5], in1=z4[:, d - 1])
                V.tensor_add(out=y[:, 5], in0=y[:, 5], in1=z4[:, d + 1])
                V.tensor_add(out=y[:, 5], in0=y[:, 5], in1=z5[:, d])
                # p6 (r2=6) = D2 z2 + D1 z5
                V.tensor_add(out=y[:, 6], in0=z2[:, d - 2], in1=z2[:, d + 2])
                V.tensor_add(out=y[:, 6], in0=y[:, 6], in1=z5[:, d - 1])
                V.tensor_add(out=y[:, 6], in0=y[:, 6], in1=z5[:, d + 1])
                # p7 (r2=8) = D2 z4 + z8
                V.tensor_add(out=y[:, 7], in0=z4[:, d - 2], in1=z4[:, d + 2])
                V.tensor_add(out=y[:, 7], in0=y[:, 7], in1=z8[:, d])
                regroup(1, 4, 8)
                # p8 (r2=9) = D2 z5 + D1 z8
                V.tensor_add(out=y[:, 8], in0=z5[:, d - 2], in1=z5[:, d + 2])
                V.tensor_add(out=y[:, 8], in0=y[:, 8], in1=z8[:, d - 1])
                V.tensor_add(out=y[:, 8], in0=y[:, 8], in1=z8[:, d + 1])
                # p9 (r2=12) = D2 z8
                V.tensor_add(out=y[:, 9], in0=z8[:, d - 2], in1=z8[:, d + 2])
                regroup(2, 8, 10)

                # ---- stage 2: K=320 matmul + bias ----
                out_sb = outp.tile([64, 2304], F32, tag="osb", name="osb")
                for (n0, nsz) in nsl:
                    ps = psp.tile([64, 512], F32, tag="ps", name="ps")
                    for c in range(3):
                        nc.tensor.matmul(
                            ps[:, :nsz],
                            w_chunks[c][:],
                            y2[c][:, n0 : n0 + nsz],
                            start=(c == 0),
                            stop=(c == 2),
                        )
                    nc.scalar.activation(
                        out_sb[:, n0 : n0 + nsz],
                        ps[:, :nsz],
                        AF.Identity,
                        bias=bias_sb[:],
                    )
                nc.sync.dma_start(out=out_d[:, dp], in_=out_sb[:])
    return nc


def _get_program():
    if "nc" not in _PROG_CACHE:
        _PROG_CACHE["nc"] = _build_program()
    return _PROG_CACHE["nc"]


def kernel(x, weight, bias, masks, _trace=False, _trace_kwargs=None):
    x = np.asarray(x, dtype=np.float32)
    weight = np.asarray(weight, dtype=np.float32)
    bias_np = np.asarray(bias, dtype=np.float32)

    # weight chunks: rows i*4+p (chunk0: p 0..3, chunk1: p 4..7), i*2+p (chunk2)
    wm = (Y0 * weight[:, :, 0, 0, 0, :]).transpose(1, 2, 0)  # [i, p, o]
    w0 = np.ascontiguousarray(wm[:, 0:4].reshape(128, 64)).astype(np.float16)
    w1 = np.ascontiguousarray(wm[:, 4:8].reshape(128, 64)).astype(np.float16)
    w2 = np.ascontiguousarray(wm[:, 8:10].reshape(64, 64)).astype(np.float16)
    bias_in = np.ascontiguousarray(bias_np.reshape(64, 1))

    xpad = np.zeros((2, 32, 52, 52, 52), dtype=np.float16)
    xpad[:, :, 2:50, 2:50, 2:50] = x[:, :, 0]

    in_maps = []
    for b in range(2):
        for dq in range(4):
            sl = np.ascontiguousarray(xpad[b, :, dq * 12 : dq * 12 + 16])
            in_maps.append(
                {"xs": sl, "w0": w0, "w1": w1, "w2": w2, "bias": bias_in}
            )

    nc = _get_program()
    bkr = run_bass_kernel_spmd(
        nc, in_maps, list(range(8)), trace=_trace, **(_trace_kwargs or {})
    )
    res = bkr.results

    out = np.empty((2, 64, 1, 48, 48, 48), dtype=np.float32)
    for core in range(8):
        b, dq = divmod(core, 4)
        out[b, :, 0, dq * 12 : (dq + 1) * 12] = res[core]["out"]
    if _trace:
        return out, bkr
    return out


# revision 30
# speedup vs baseline: 1.1142x; 1.1142x over previous
"""Trainium2 Bass kernel for EquiLocalPatOrientConvolution.

Reference computes: filt[o,i,·] = Y0 * sum_p W[o,i,p] * radial_mask_p (5x5x5),
then conv3d(x, filt, pad=2) + bias.

Factorized device algorithm (11x fewer MACs than direct conv):
  stage 1 (DVE, fp16): per input channel, radial-shell sums of x via a
    separable symmetric-shift hierarchy (w-axis sums -> 2D shells over (h,w)
    -> 3D shells), ~31 ops/point instead of 125 taps.
  stage 2 (PE): out[o,n] = sum_{i,p} (Y0*W[o,i,p]) * y[i,p,n] as K=320
    contraction (3 PSUM-accumulated matmuls), bias fused in the PSUM->SBUF
    activation copy.

Sharding: 8 cores = (batch 2) x (depth quarters 4); each core gets a
zero-padded x slab [32, 16, 52, 52] (12 depth + 2+2 halo, h/w padded).
"""

import numpy as np

import concourse.bacc as bacc
import concourse.mybir as mybir
from concourse.tile import TileContext
from concourse.bass_utils import run_bass_kernel_spmd

Y0 = 0.28209479177387814
F16 = mybir.dt.float16
F32 = mybir.dt.float32

_PROG_CACHE = {}


def _build_program():
    nc = bacc.Bacc(None)
    xs = nc.declare_dram_parameter("xs", [32, 16, 52, 52], F16, isOutput=False)
    w0 = nc.declare_dram_parameter("w0", [128, 64], F16, isOutput=False)
    w1 = nc.declare_dram_parameter("w1", [128, 64], F16, isOutput=False)
    w2 = nc.declare_dram_parameter("w2", [64, 64], F16, isOutput=False)
    bias = nc.declare_dram_parameter("bias", [64, 1], F32, isOutput=False)
    out_d = nc.declare_dram_parameter("out", [64, 12, 48, 48], F32, isOutput=True)

    V = nc.vector
    AF = mybir.ActivationFunctionType

    with TileContext(nc) as tc:
        with (
            tc.tile_pool(name="persist", bufs=1) as pp,
            tc.tile_pool(name="pspool", bufs=6, space="PSUM") as psp,
        ):
            # ---- load x slab: partitions = (hc, i) = hc*32 + i ----
            x_sb = pp.tile([128, 16, 16, 52], F16)
            for hc in range(4):
                eng = nc.sync if hc % 2 == 0 else nc.scalar
                eng.dma_start(
                    out=x_sb[hc * 32 : (hc + 1) * 32],
                    in_=xs[:, :, hc * 12 : hc * 12 + 16, :],
                )
            w_sb0 = pp.tile([128, 64], F16)
            w_sb1 = pp.tile([128, 64], F16)
            w_sb2 = pp.tile([64, 64], F16)
            nc.sync.dma_start(out=w_sb0[:], in_=w0[:])
            nc.sync.dma_start(out=w_sb1[:], in_=w1[:])
            nc.sync.dma_start(out=w_sb2[:], in_=w2[:])
            w_chunks = [w_sb0, w_sb1, w_sb2]
            bias_sb = pp.tile([64, 1], F32)
            nc.sync.dma_start(out=bias_sb[:], in_=bias[:])

            # z tiles outlive the u scope below
            z1 = pp.tile([128, 16, 12, 48], F16)
            z2 = pp.tile([128, 16, 12, 48], F16)
            z4 = pp.tile([128, 16, 12, 48], F16)
            z5 = pp.tile([128, 16, 12, 48], F16)
            z8 = pp.tile([128, 16, 12, 48], F16)

            # ---- stage 1a/1b in a transient pool scope (u freed after) ----
            with tc.tile_pool(name="upool", bufs=1) as up:
                # w-axis symmetric sums u_s = x(w-s) + x(w+s)
                u1 = up.tile([128, 16, 16, 48], F16)
                u2 = up.tile([128, 16, 16, 48], F16)
                V.tensor_add(out=u1[:], in0=x_sb[:, :, :, 1:49], in1=x_sb[:, :, :, 3:51])
                V.tensor_add(out=u2[:], in0=x_sb[:, :, :, 0:48], in1=x_sb[:, :, :, 4:52])

                # 2D shells over (h,w): z_A, A in {0,1,2,4,5,8}
                # z0 = x (view); others at h-interior [2,14), w-interior
                V.tensor_add(out=z1[:], in0=x_sb[:, :, 1:13, 2:50], in1=x_sb[:, :, 3:15, 2:50])
                V.tensor_add(out=z1[:], in0=z1[:], in1=u1[:, :, 2:14, :])
                V.tensor_add(out=z2[:], in0=u1[:, :, 1:13, :], in1=u1[:, :, 3:15, :])
                V.tensor_add(out=z4[:], in0=x_sb[:, :, 0:12, 2:50], in1=x_sb[:, :, 4:16, 2:50])
                V.tensor_add(out=z4[:], in0=z4[:], in1=u2[:, :, 2:14, :])
                V.tensor_add(out=z5[:], in0=u1[:, :, 0:12, :], in1=u1[:, :, 4:16, :])
                V.tensor_add(out=z5[:], in0=z5[:], in1=u2[:, :, 1:13, :])
                V.tensor_add(out=z5[:], in0=z5[:], in1=u2[:, :, 3:15, :])
                V.tensor_add(out=z8[:], in0=u2[:, :, 0:12, :], in1=u2[:, :, 4:16, :])

            def x0(d):
                return x_sb[:, d, 2:14, 2:50]

            nsl = [(0, 512), (512, 512), (1024, 512), (1536, 512), (2048, 256)]

            # ---- per output-depth plane: 3D shells -> regroup -> matmul ----
            with (
                tc.tile_pool(name="ypool", bufs=3) as yp,
                tc.tile_pool(name="y2pool", bufs=2) as y2p,
                tc.tile_pool(name="outpool", bufs=1) as outp,
            ):
                _run_planes(nc, V, x0, x_sb, z1, z2, z4, z5, z8, yp, y2p, psp, outp,
                            w_chunks, bias_sb, out_d, nsl)
    # Bacc finalize runs generate_event_semaphores: splits embedded waits
    # into standalone EventSemaphore insts (walrus rejects >1 wait per inst)
    nc.finalize()
    return nc


def _run_planes(nc, V, x0, x_sb, z1, z2, z4, z5, z8, yp, y2p, psp, outp,
                w_chunks, bias_sb, out_d, nsl):
    AF = mybir.ActivationFunctionType
    if True:
        if True:
            for dp in range(12):
                d = dp + 2
                y = yp.tile([128, 10, 12, 48], F16, tag="y")
                y2 = [
                    y2p.tile([128, 2304], F16, tag="y2_0", name="y2_0"),
                    y2p.tile([128, 2304], F16, tag="y2_1", name="y2_1"),
                    y2p.tile([64, 2304], F16, tag="y2_2", name="y2_2"),
                ]

                def regroup(c, plo, phi):
                    # [(hc,i), (p,h,w)] -> K-chunk layout [(i,p), n];
                    # chunk rows = i*4 + p_local (c<2), i*2 + p_local (c=2);
                    # free n = (hc, h, w) flattened to 2304. Issued right
                    # after the producing ops so chunk-c matmuls overlap
                    # the DVE work of later shells.
                    for hc in range(4):
                        nc.sync.dma_start(
                            out=y2[c][:, hc * 576 : (hc + 1) * 576],
                            in_=y[hc * 32 : (hc + 1) * 32, plo:phi],
                        )

                # p0 (r2=0) on ScalarE (idle) to keep DVE for the adds
                nc.scalar.activation(y[:, 0], x0(d), AF.Copy)
                # p1 (r2=1) = D1 z0 + z1
                V.tensor_add(out=y[:, 1], in0=x0(d - 1), in1=x0(d + 1))
                V.tensor_add(out=y[:, 1], in0=y[:, 1], in1=z1[:, d])
                # p2 (r2=2) = D1 z1 + z2
                V.tensor_add(out=y[:, 2], in0=z1[:, d - 1], in1=z1[:, d + 1])
                V.tensor_add(out=y[:, 2], in0=y[:, 2], in1=z2[:, d])
                # p3 (r2=3) = D1 z2
                V.tensor_add(out=y[:, 3], in0=z2[:, d - 1], in1=z2[:, d + 1])
                regroup(0, 0, 4)
                # p4 (r2=4) = D2 z0 + z4
                V.tensor_add(out=y[:, 4], in0=x0(d - 2), in1=x0(d + 2))
                V.tensor_add(out=y[:, 4], in0=y[:, 4], in1=z4[:, d])
                # p5 (r2=5) = D2 z1 + D1 z4 + z5
                V.tensor_add(out=y[:, 5], in0=z1[:, d - 2], in1=z1[:, d + 2])
                V.tensor_add(out=y[:, 5], in0=y[:, 5], in1=z4[:, d - 1])
                V.tensor_add(out=y[:, 5], in0=y[:, 5], in1=z4[:, d + 1])
                V.tensor_add(out=y[:, 5], in0=y[:, 5], in1=z5[:, d])
                # p6 (r2=6) = D2 z2 + D1 z5
                V.tensor_add(out=y[:, 6], in0=z2[:, d - 2], in1=z2[:, d + 2])
                V.tensor_add(out=y[:, 6], in0=y[:, 6], in1=z5[:, d - 1])
                V.tensor_add(out=y[:, 6], in0=y[:, 6], in1=z5[:, d + 1])
                # p7 (r2=8) = D2 z4 + z8
                V.tensor_add(out=y[:, 7], in0=z4[:, d - 2], in1=z4[:, d + 2])
                V.tensor_add(out=y[:, 7], in0=y[:, 7], in1=z8[:, d])
                regroup(1, 4, 8)
                # p8 (r2=9) = D2 z5 + D1 z8
                V.tensor_add(out=y[:, 8], in0=z5[:, d - 2], in1=z5[:, d + 2])
                V.tensor_add(out=y[:, 8], in0=y[:, 8], in1=z8[:, d - 1])
                V.tensor_add(out=y[:, 8], in0=y[:, 8], in1=z8[:, d + 1])
                # p9 (r2=12) = D2 z8
                V.tensor_add(out=y[:, 9], in0=z8[:, d - 2], in1=z8[:, d + 2])
                regroup(2, 8, 10)

                # ---- stage 2: K=320 matmul + bias ----
                out_sb = outp.tile([64, 2304], F32, tag="osb", name="osb")
                for (n0, nsz) in nsl:
                    ps = psp.tile([64, 512], F32, tag="ps", name="ps")
                    for c in range(3):
                        nc.tensor.matmul(
                            ps[:, :nsz],
                            w_chunks[c][:],
                            y2[c][:, n0 : n0 + nsz],
                            start=(c == 0),
                            stop=(c == 2),
                        )
                    nc.scalar.activation(
                        out_sb[:, n0 : n0 + nsz],
                        ps[:, :nsz],
                        AF.Identity,
                        bias=bias_sb[:],
                    )
                nc.sync.dma_start(out=out_d[:, dp], in_=out_sb[:])
    return nc


def _get_program():
    if "nc" not in _PROG_CACHE:
        _PROG_CACHE["nc"] = _build_program()
    return _PROG_CACHE["nc"]


def kernel(x, weight, bias, masks, _trace=False, _trace_kwargs=None):
    x = np.asarray(x, dtype=np.float32)
    weight = np.asarray(weight, dtype=np.float32)
    bias_np = np.asarray(bias, dtype=np.float32)

    # weight chunks: rows i*4+p (chunk0: p 0..3, chunk1: p 4..7), i*2+p (chunk2)
    wm = (Y0 * weight[:, :, 0, 0, 0, :]).transpose(1, 2, 0)  # [i, p, o]
    w0 = np.ascontiguousarray(wm[:, 0:4].reshape(128, 64)).astype(np.float16)
    w1 = np.ascontiguousarray(wm[:, 4:8].reshape(128, 64)).astype(np.float16)
    w2 = np.ascontiguousarray(wm[:, 8:10].reshape(64, 64)).astype(np.float16)
    bias_in = np.ascontiguousarray(bias_np.reshape(64, 1))

    xpad = np.zeros((2, 32, 52, 52, 52), dtype=np.float16)
    xpad[:, :, 2:50, 2:50, 2:50] = x[:, :, 0]

    in_maps = []
    for b in range(2):
        for dq in range(4):
            sl = np.ascontiguousarray(xpad[b, :, dq * 12 : dq * 12 + 16])
            in_maps.append(
                {"xs": sl, "w0": w0, "w1": w1, "w2": w2, "bias": bias_in}
            )

    nc = _get_program()
    bkr = run_bass_kernel_spmd(
        nc, in_maps, list(range(8)), trace=_trace, **(_trace_kwargs or {})
    )
    res = bkr.results

    out = np.empty((2, 64, 1, 48, 48, 48), dtype=np.float32)
    for core in range(8):
        b, dq = divmod(core, 4)
        out[b, :, 0, dq * 12 : (dq + 1) * 12] = res[core]["out"]
    if _trace:
        return out, bkr
    return out
